# revision 12
# baseline (speedup 1.0000x reference)
"""Trainium2 Bass kernel for nn_MCA_12214886990440 (strip-conv dual-axis attention).

Sharding: data-parallel over batch B=8 across 8 NeuronCores (params replicated).

Per-core math (one batch element, C=64, H=W=128, NH=8, D=8):
  xh = bn1(x); xw = bn2(x)
  sc_h = multi-k strip conv along H (3 kernels presummed into one 21-tap conv)
  sc_w = strip conv along W
  qkv (conv bias folded into qkv bias); attention per head reassociated:
      w_o^T = wk_t @ G_w,  G_w = sum_d hq(d)^T wv(d)   [128x128 Gram]
      h_o^T = hk_t @ G_h,  G_h = sum_d wq(d)^T hv(d)
  y = x * sigmoid(wout@w_o + hout@h_o + b)

All matmuls run as float32r (full-rate fp32). Layout pivots (channel-partition
<-> pixel-partition) go through small internal DRAM tensors ("cp parking").
"""
import sys
sys.path.insert(0, "/opt/trn_rl_repo")

import numpy as np

import concourse.bass as bass
import concourse.tile as tile
from concourse import bacc
from concourse import mybir

B, C, H, W, NH, D = 8, 64, 128, 128, 8, 8
KS = [7, 11, 21]
EPS = 1e-5
PAD = 10          # max k//2
NTAP = 21
HW = H * W        # 16384
PADROWS = H + 2 * PAD  # 148
F32 = mybir.dt.float32
F32R = mybir.dt.float32r
AF = mybir.ActivationFunctionType
ALU = mybir.AluOpType

N_CORES = 8
NCHUNK = 32       # pixel chunks of 512
CH = 512


def _r(ap):
    return ap.bitcast(F32R)


def _kernel_body(tc, a):
    nc = tc.nc

    # ---------------- persistent pools ----------------
    dp = tc.alloc_tile_pool(name="dram", bufs=1, space="DRAM")
    wp = tc.alloc_tile_pool(name="wts", bufs=1)
    xcp = tc.alloc_tile_pool(name="xc", bufs=2)
    evp = tc.alloc_tile_pool(name="evac", bufs=2)

    # packed weights: two tiles (column layouts must match _prep_weights)
    w128 = wp.tile([128, 1478], F32R, tag="w128", name="w128")
    nc.sync.dma_start(w128[:], a["w128"])
    w64 = wp.tile([64, 387], F32R, tag="w64", name="w64")
    nc.sync.dma_start(w64[:], a["w64"])
    convw = [w128[:, 0:704], w128[:, 704:1408]]
    projw = w128[:, 1408:1472]
    qkv1b = [w128[:, 1472:1473].bitcast(F32), w128[:, 1473:1474].bitcast(F32)]
    bnab = w128[:, 1474:1478].bitcast(F32)
    qkv1 = [w64[:, 0:128], w64[:, 128:256]]
    qkv2 = [w64[:, 256:320], w64[:, 320:384]]
    qkv2b = [w64[:, 384:385].bitcast(F32), w64[:, 385:386].bitcast(F32)]
    projb = w64[:, 386:387].bitcast(F32)

    # internal DRAM parking for the channel->pixel pivot
    cp_qv = [dp.tile([128, HW], F32, tag=f"cp_qv{i}", name=f"cp_qv{i}") for i in range(2)]
    cp_k = [dp.tile([64, HW], F32, tag=f"cp_k{i}", name=f"cp_k{i}") for i in range(2)]

    # ---------------- phase 1: BN + convs + qkv ----------------
    scp = tc.alloc_tile_pool(name="sc", bufs=1)
    pp = tc.alloc_tile_pool(name="pad", bufs=1)
    ps_conv = tc.alloc_tile_pool(name="ps_conv", bufs=4, space="PSUM")
    ps_qkv1 = tc.alloc_tile_pool(name="ps_qkv1", bufs=2, space="PSUM")
    ps_qkv2 = tc.alloc_tile_pool(name="ps_qkv2", bufs=2, space="PSUM")

    for br in range(2):  # 0 = h-branch (conv along H), 1 = w-branch (conv along W)
        scb = scp.tile([64, HW], F32R, tag="sc", name=f"sc{br}")
        pad = pp.tile([128, PADROWS * W], F32R, tag="pad")
        prr = pad[:].rearrange("p (h j) -> p h j", j=PADROWS)
        if br == 0:
            # h-major rows of W; pad rows top/bottom; parts 64-127 = 1-row shift
            nc.vector.memset(pad[0:64, 0:PAD * W].bitcast(F32), 0.0)
            nc.vector.memset(pad[0:64, (H + PAD) * W:].bitcast(F32), 0.0)
            nc.vector.memset(pad[64:128, 0:(PAD - 1) * W].bitcast(F32), 0.0)
            nc.vector.memset(pad[64:128, (H + PAD - 1) * W:].bitcast(F32), 0.0)
        else:
            # rows of length 148 (w-padded); parts 64-127 = 1-col shift
            nc.vector.memset(prr[0:64, :, 0:PAD].bitcast(F32), 0.0)
            nc.vector.memset(prr[0:64, :, H + PAD:].bitcast(F32), 0.0)
            nc.vector.memset(prr[64:128, :, 0:PAD - 1].bitcast(F32), 0.0)
            nc.vector.memset(prr[64:128, :, H + PAD - 1:].bitcast(F32), 0.0)

        # BatchNorm (affine) into the padded buffer, 4 h-rows per chunk
        av0 = bnab[0:64, 2 * br:2 * br + 1]
        bv0 = bnab[0:64, 2 * br + 1:2 * br + 2]
        av1 = bnab[64:128, 2 * br:2 * br + 1]
        bv1 = bnab[64:128, 2 * br + 1:2 * br + 2]
        for i in range(NCHUNK):
            xc = xcp.tile([128, CH], F32, tag="xc")
            src = a["x"][:, i * CH:(i + 1) * CH]
            nc.sync.dma_start(xc[0:64, :], src)
            nc.sync.dma_start(xc[64:128, :], src)
            if br == 0:
                d0 = pad[0:64, (PAD + 4 * i) * W:(PAD + 4 * i) * W + CH]
                d1 = pad[64:128, (PAD - 1 + 4 * i) * W:(PAD - 1 + 4 * i) * W + CH]
                s0, s1 = xc[0:64, :], xc[64:128, :]
            else:
                d0 = prr[0:64, 4 * i:4 * i + 4, PAD:PAD + W]
                d1 = prr[64:128, 4 * i:4 * i + 4, PAD - 1:PAD - 1 + W]
                s0 = xc[0:64, :].rearrange("p (h w) -> p h w", w=W)
                s1 = xc[64:128, :].rearrange("p (h w) -> p h w", w=W)
            nc.vector.tensor_scalar(d0, s0, av0, bv0, ALU.mult, ALU.add)
            nc.vector.tensor_scalar(d1, s1, av1, bv1, ALU.mult, ALU.add)

        # conv: per 512-px chunk, 10 tap-pair matmuls (K=128) + 1 single (K=64)
        cw = convw[br]
        for ci in range(NCHUNK):
            ps = ps_conv.tile([64, CH], F32, tag="conv")
            for g in range(10):
                if br == 0:
                    rhs = pad[:, (4 * ci + 2 * g) * W:(4 * ci + 2 * g) * W + CH]
                else:
                    rhs = prr[:, 4 * ci:4 * ci + 4, 2 * g:2 * g + W]
                nc.tensor.matmul(ps[:], cw[:, g * 64:(g + 1) * 64], rhs,
                                 start=(g == 0), stop=False)
            if br == 0:
                rhs = pad[0:64, (4 * ci + 20) * W:(4 * ci + 20) * W + CH]
            else:
                rhs = prr[0:64, 4 * ci:4 * ci + 4, 20:20 + W]
            nc.tensor.matmul(ps[:], cw[0:64, 640:704], rhs,
                             start=False, stop=True)
            nc.scalar.activation(scb[:, ci * CH:(ci + 1) * CH], ps[:], AF.Copy)

        # qkv pass 1: q|v (M=128), contiguous chunks -> cp_qv, (h,w)-major rows
        for ci in range(NCHUNK):
            ps = ps_qkv1.tile([128, CH], F32, tag="qkv1")
            nc.tensor.matmul(ps[:], qkv1[br],
                             scb[:, ci * CH:(ci + 1) * CH],
                             start=True, stop=True)
            ev = evp.tile([128, CH], F32, tag="ev")
            nc.scalar.activation(ev[:], ps[:], AF.Identity, bias=qkv1b[br])
            nc.sync.dma_start(cp_qv[br][:, ci * CH:(ci + 1) * CH], ev[:])

        # qkv pass 2: k (M=64), w-column chunks -> cp_k, (w,h)-major rows
        scr = scb[:].rearrange("p (h w) -> p w h", w=W)
        for ci in range(NCHUNK):
            ps = ps_qkv2.tile([64, CH], F32, tag="qkv2")
            nc.tensor.matmul(ps[:], qkv2[br],
                             scr[:, 4 * ci:4 * ci + 4, :],
                             start=True, stop=True)
            ev = evp.tile([128, CH], F32, tag="ev", name="ev2")[0:64, :]
            nc.scalar.activation(ev[:], ps[:], AF.Identity, bias=qkv2b[br])
            nc.sync.dma_start(cp_k[br][:, ci * CH:(ci + 1) * CH], ev[:])

    # release phase-1 pools (LIFO per space)
    ps_qkv2.release()
    ps_qkv1.release()
    ps_conv.release()
    pp.release()
    scp.release()

    # ---------------- phase 2: attention + projection ----------------
    scp2 = tc.alloc_tile_pool(name="scp2", bufs=1)
    s_cp = scp2.tile([128, HW], F32R, tag="s_cp")
    gsb = tc.alloc_tile_pool(name="gsb", bufs=1)
    g_sb = gsb.tile([128, 16 * 128], F32, tag="g_sb")
    sprq = tc.alloc_tile_pool(name="sprq", bufs=4)
    sprv = tc.alloc_tile_pool(name="sprv", bufs=4)
    sprk = tc.alloc_tile_pool(name="sprk", bufs=4)
    btev = tc.alloc_tile_pool(name="btev", bufs=2)
    sigp = tc.alloc_tile_pool(name="sigp", bufs=2)
    outp = tc.alloc_tile_pool(name="outp", bufs=2)
    ps_g = tc.alloc_tile_pool(name="ps_g", bufs=2, space="PSUM")
    ps_bt = tc.alloc_tile_pool(name="ps_bt", bufs=4, space="PSUM")
    ps_pj = tc.alloc_tile_pool(name="ps_pj", bufs=2, space="PSUM")

    # G matrices: gi=0 -> G_w = sum hq^T wv ; gi=1 -> G_h = sum wq^T hv
    for gi in range(2):
        qsrc = cp_qv[0] if gi == 0 else cp_qv[1]   # q lives in rows 0..64
        vsrc = cp_qv[1] if gi == 0 else cp_qv[0]   # v lives in rows 64..128
        for nh in range(NH):
            gps = ps_g.tile([128, 128], F32, tag="g")
            for d in range(D):
                c = nh * D + d
                qa = sprq.tile([128, 128], F32, tag="q")
                nc.sync.dma_start(
                    qa[:], qsrc[c:c + 1, :].rearrange("o (h w) -> o h w", w=W))
                va = sprv.tile([128, 128], F32, tag="v")
                nc.sync.dma_start(
                    va[:], vsrc[64 + c:65 + c, :].rearrange("o (h w) -> o h w", w=W))
                nc.tensor.matmul(gps[:], qa[:], va[:],
                                 start=(d == 0), stop=(d == D - 1))
            nc.scalar.activation(
                g_sb[:, (gi * NH + nh) * 128:(gi * NH + nh + 1) * 128],
                gps[:], AF.Copy)

    # B^T: per (branch, head, d): psum = k_t(nh,d) @ G -> natural [H,W] image,
    # reverse-spread into channel-partition S_cp (w_o rows 0-63, h_o rows 64-127)
    for gi in range(2):
        ksrc = cp_k[1] if gi == 0 else cp_k[0]  # w_o uses wk; h_o uses hk
        for nh in range(NH):
            gref = g_sb[:, (gi * NH + nh) * 128:(gi * NH + nh + 1) * 128]
            for d in range(D):
                c = nh * D + d
                ka = sprk.tile([128, 128], F32, tag="k")
                nc.sync.dma_start(
                    ka[:], ksrc[c:c + 1, :].rearrange("o (w h) -> o w h", h=H))
                bps = ps_bt.tile([128, 128], F32, tag="bt")
                nc.tensor.matmul(bps[:], ka[:], gref, start=True, stop=True)
                bt = btev.tile([128, 128], F32R, tag="btv")
                nc.scalar.activation(bt[:], bps[:], AF.Copy)
                nc.sync.dma_start(
                    s_cp[gi * 64 + c:gi * 64 + c + 1, :].rearrange(
                        "o (h w) -> o h w", w=W),
                    bt[:])

    # collapse the 128 reverse-spread DMA deps into one sync point so the
    # first projection matmul doesn't exceed the per-instruction wait limit
    tc.strict_bb_all_engine_barrier()

    # fused output projection (both branches, K=128) + sigmoid + x*sig -> y
    for ci in range(NCHUNK):
        pps = ps_pj.tile([64, CH], F32, tag="pj")
        nc.tensor.matmul(pps[:], projw, s_cp[:, ci * CH:(ci + 1) * CH],
                         start=True, stop=True)
        sg = sigp.tile([64, CH], F32, tag="sg")
        nc.scalar.activation(sg[:], pps[:], AF.Sigmoid, bias=projb)
        xc = outp.tile([64, CH], F32, tag="xm")
        nc.sync.dma_start(xc[:], a["x"][:, ci * CH:(ci + 1) * CH])
        ot = outp.tile([64, CH], F32, tag="ot")
        nc.vector.tensor_mul(ot[:], sg[:], xc[:])
        nc.sync.dma_start(a["y"][:, ci * CH:(ci + 1) * CH], ot[:])

    for p in (ps_pj, ps_bt, ps_g, outp, sigp, btev, sprk, sprv, sprq,
              gsb, scp2, evp, xcp, wp, dp):
        p.release()


def _prep_weights(inputs):
    """Host-side packing: BN affine, presummed conv taps, folded qkv biases."""
    inp = {k: np.asarray(v, dtype=np.float64) for k, v in inputs.items()}
    w = {}
    a1 = inp["bn1_g"] / np.sqrt(inp["bn1_v"] + EPS)
    b1 = inp["bn1_b"] - inp["bn1_m"] * a1
    a2 = inp["bn2_g"] / np.sqrt(inp["bn2_v"] + EPS)
    b2 = inp["bn2_b"] - inp["bn2_m"] * a2
    w["bnab"] = np.tile(np.stack([a1, b1, a2, b2], axis=1), (2, 1))  # [128, 4]

    def conv_pack(ws):
        eff = np.zeros((NTAP, C, C))
        for j, k in enumerate(KS):
            off = PAD - k // 2
            for i in range(k):
                eff[off + i] += ws[j][:, :, i]
        pk = np.zeros((128, 704))
        for g in range(10):
            pk[0:64, g * 64:(g + 1) * 64] = eff[2 * g].T
            pk[64:128, g * 64:(g + 1) * 64] = eff[2 * g + 1].T
        pk[0:64, 640:704] = eff[20].T
        return pk

    w["convw_h"] = conv_pack([inp[f"sc1_w{j}"][:, :, :, 0] for j in range(3)])
    w["convw_w"] = conv_pack([inp[f"sc2_w{j}"][:, :, 0, :] for j in range(3)])
    bch = inp["sc1_b0"] + inp["sc1_b1"] + inp["sc1_b2"]
    bcw = inp["sc2_b0"] + inp["sc2_b1"] + inp["sc2_b2"]

    scale = D * H ** (-0.5)
    idx = (np.arange(NH)[:, None] * 24 + np.arange(D)[None, :]).ravel()
    idx_q, idx_k, idx_v = idx, idx + 8, idx + 16

    for br, (qw, qb, bc) in enumerate(
            [(inp["hqkv_w"], inp["hqkv_b"], bch),
             (inp["wqkv_w"], inp["wqkv_b"], bcw)]):
        bfold = qb + qw @ bc
        Wq, Wk, Wv = qw[idx_q] * scale, qw[idx_k], qw[idx_v]
        bq, bk, bv = bfold[idx_q] * scale, bfold[idx_k], bfold[idx_v]
        sfx = "h" if br == 0 else "w"
        w[f"qkv1_{sfx}"] = np.concatenate([Wq.T, Wv.T], axis=1)        # [64,128]
        w[f"qkv1b_{sfx}"] = np.concatenate([bq, bv])[:, None]          # [128,1]
        w[f"qkv2_{sfx}"] = Wk.T                                        # [64,64]
        w[f"qkv2b_{sfx}"] = bk[:, None]                                # [64,1]

    w["projw"] = np.concatenate([inp["wout_w"].T, inp["hout_w"].T], axis=0)  # [128,64]
    w["projb"] = (inp["wout_b"] + inp["hout_b"])[:, None]                    # [64,1]

    w128 = np.zeros((128, 1478))
    w128[:, 0:704] = w["convw_h"]
    w128[:, 704:1408] = w["convw_w"]
    w128[:, 1408:1472] = w["projw"]
    w128[:, 1472:1473] = w["qkv1b_h"]
    w128[:, 1473:1474] = w["qkv1b_w"]
    w128[:, 1474:1478] = w["bnab"]
    w64 = np.zeros((64, 387))
    w64[:, 0:128] = w["qkv1_h"]
    w64[:, 128:256] = w["qkv1_w"]
    w64[:, 256:320] = w["qkv2_h"]
    w64[:, 320:384] = w["qkv2_w"]
    w64[:, 384:385] = w["qkv2b_h"]
    w64[:, 385:386] = w["qkv2b_w"]
    w64[:, 386:387] = w["projb"]
    return {"w128": _to_f32r(w128), "w64": _to_f32r(w64)}


_NC_CACHE = {}
_RUN_OPTS = {"trace": False}
_LAST_RESULT = {}

_W_SHAPES = {"x": [C, HW], "w128": [128, 1478], "w64": [64, 387]}
_W_DTYPES = {"x": F32, "w128": F32R, "w64": F32R}


def _to_f32r(a):
    """fp32 -> fp32r: round mantissa to 11 bits (top 20 bits kept)."""
    u = np.ascontiguousarray(a, dtype=np.float32).view(np.uint32).astype(np.uint64)
    u = (u + 0x800) & np.uint64(0xFFFFF000)
    return u.astype(np.uint32).view(np.float32)


def _build_nc():
    if "nc" in _NC_CACHE:
        return _NC_CACHE["nc"]
    nc = bacc.Bacc(trn_type="TRN2", target_bir_lowering=False, debug=False)
    a = {}
    for n, s in _W_SHAPES.items():
        a[n] = nc.dram_tensor(n, s, _W_DTYPES[n], kind="ExternalInput").ap()
    a["y"] = nc.dram_tensor("y", [C, HW], F32, kind="ExternalOutput").ap()
    with tile.TileContext(nc) as tc:
        _kernel_body(tc, a)
    nc.compile()
    _NC_CACHE["nc"] = nc
    return nc


def _in_maps(inputs):
    w = _prep_weights(inputs)
    x = np.ascontiguousarray(np.asarray(inputs["x"], dtype=np.float32))
    maps = []
    for core in range(N_CORES):
        m = {"x": np.ascontiguousarray(x[core].reshape(C, HW))}
        m.update(w)
        maps.append(m)
    return maps


def kernel(**inputs):
    from concourse.bass_utils import run_bass_kernel_spmd

    nc = _build_nc()
    res = run_bass_kernel_spmd(nc, _in_maps(inputs), core_ids=list(range(N_CORES)),
                               trace=_RUN_OPTS["trace"])
    _LAST_RESULT["res"] = res
    out = np.stack([res.results[i]["y"].reshape(C, H, W) for i in range(N_CORES)])
    return out.astype(np.float32)


if __name__ == "__main__":
    nc = _build_nc()
    print("built ok")


# revision 13
# speedup vs baseline: 1.1531x; 1.1531x over previous
"""Trainium2 Bass kernel for nn_MCA_12214886990440 (strip-conv dual-axis attention).

Sharding: data-parallel over batch B=8 across 8 NeuronCores (params replicated).

Per-core math (one batch element, C=64, H=W=128, NH=8, D=8):
  xh = bn1(x); xw = bn2(x)
  sc_h = multi-k strip conv along H (3 kernels presummed into one 21-tap conv)
  sc_w = strip conv along W
  qkv (conv bias folded into qkv bias); attention per head reassociated:
      w_o^T = wk_t @ G_w,  G_w = sum_d hq(d)^T wv(d)   [128x128 Gram]
      h_o^T = hk_t @ G_h,  G_h = sum_d wq(d)^T hv(d)
  y = x * sigmoid(wout@w_o + hout@h_o + b)

All matmuls run as float32r (full-rate fp32). Layout pivots (channel-partition
<-> pixel-partition) go through small internal DRAM tensors ("cp parking").
"""
import sys
sys.path.insert(0, "/opt/trn_rl_repo")

import numpy as np

import concourse.bass as bass
import concourse.tile as tile
from concourse import bacc
from concourse import mybir

B, C, H, W, NH, D = 8, 64, 128, 128, 8, 8
KS = [7, 11, 21]
EPS = 1e-5
PAD = 10          # max k//2
NTAP = 21
HW = H * W        # 16384
PADROWS = H + 2 * PAD  # 148
F32 = mybir.dt.float32
F32R = mybir.dt.float32r
AF = mybir.ActivationFunctionType
ALU = mybir.AluOpType

N_CORES = 8
NCHUNK = 32       # pixel chunks of 512
CH = 512


def _r(ap):
    return ap.bitcast(F32R)


def _kernel_body(tc, a, reps=1):
    nc = tc.nc
    for _rep in range(reps):
        _one_pass(tc, a)


def _one_pass(tc, a):
    nc = tc.nc

    # ---------------- persistent pools ----------------
    dp = tc.alloc_tile_pool(name="dram", bufs=1, space="DRAM")
    wp = tc.alloc_tile_pool(name="wts", bufs=1)
    xcp = tc.alloc_tile_pool(name="xc", bufs=2)
    evp = tc.alloc_tile_pool(name="evac", bufs=2)

    # packed weights: two tiles (column layouts must match _prep_weights)
    w128 = wp.tile([128, 1478], F32R, tag="w128", name="w128")
    nc.sync.dma_start(w128[:], a["w128"])
    w64 = wp.tile([64, 387], F32R, tag="w64", name="w64")
    nc.sync.dma_start(w64[:], a["w64"])
    convw = [w128[:, 0:704], w128[:, 704:1408]]
    projw = w128[:, 1408:1472]
    qkv1b = [w128[:, 1472:1473].bitcast(F32), w128[:, 1473:1474].bitcast(F32)]
    bnab = w128[:, 1474:1478].bitcast(F32)
    qkv1 = [w64[:, 0:128], w64[:, 128:256]]
    qkv2 = [w64[:, 256:320], w64[:, 320:384]]
    qkv2b = [w64[:, 384:385].bitcast(F32), w64[:, 385:386].bitcast(F32)]
    projb = w64[:, 386:387].bitcast(F32)

    # internal DRAM parking for the channel->pixel pivot
    cp_qv = [dp.tile([128, HW], F32, tag=f"cp_qv{i}", name=f"cp_qv{i}") for i in range(2)]
    cp_k = [dp.tile([64, HW], F32, tag=f"cp_k{i}", name=f"cp_k{i}") for i in range(2)]

    # ---------------- phase 1: BN + convs + qkv ----------------
    scp = tc.alloc_tile_pool(name="sc", bufs=1)
    pp = tc.alloc_tile_pool(name="pad", bufs=1)
    ps_conv = tc.alloc_tile_pool(name="ps_conv", bufs=4, space="PSUM")
    ps_qkv1 = tc.alloc_tile_pool(name="ps_qkv1", bufs=2, space="PSUM")
    ps_qkv2 = tc.alloc_tile_pool(name="ps_qkv2", bufs=2, space="PSUM")

    for br in range(2):  # 0 = h-branch (conv along H), 1 = w-branch (conv along W)
        scb = scp.tile([64, HW], F32R, tag="sc", name=f"sc{br}")
        pad = pp.tile([128, PADROWS * W], F32R, tag="pad")
        prr = pad[:].rearrange("p (h j) -> p h j", j=PADROWS)
        if br == 0:
            # h-major rows of W; pad rows top/bottom; parts 64-127 = 1-row shift
            nc.vector.memset(pad[0:64, 0:PAD * W].bitcast(F32), 0.0)
            nc.vector.memset(pad[0:64, (H + PAD) * W:].bitcast(F32), 0.0)
            nc.vector.memset(pad[64:128, 0:(PAD - 1) * W].bitcast(F32), 0.0)
            nc.vector.memset(pad[64:128, (H + PAD - 1) * W:].bitcast(F32), 0.0)
        else:
            # rows of length 148 (w-padded); parts 64-127 = 1-col shift
            nc.vector.memset(prr[0:64, :, 0:PAD].bitcast(F32), 0.0)
            nc.vector.memset(prr[0:64, :, H + PAD:].bitcast(F32), 0.0)
            nc.vector.memset(prr[64:128, :, 0:PAD - 1].bitcast(F32), 0.0)
            nc.vector.memset(prr[64:128, :, H + PAD - 1:].bitcast(F32), 0.0)

        # BatchNorm (affine) into the padded buffer, 4 h-rows per chunk
        av0 = bnab[0:64, 2 * br:2 * br + 1]
        bv0 = bnab[0:64, 2 * br + 1:2 * br + 2]
        av1 = bnab[64:128, 2 * br:2 * br + 1]
        bv1 = bnab[64:128, 2 * br + 1:2 * br + 2]
        for i in range(NCHUNK):
            xc = xcp.tile([128, CH], F32, tag="xc")
            src = a["x"][:, i * CH:(i + 1) * CH]
            nc.sync.dma_start(xc[0:64, :], src)
            nc.sync.dma_start(xc[64:128, :], src)
            if br == 0:
                d0 = pad[0:64, (PAD + 4 * i) * W:(PAD + 4 * i) * W + CH]
                d1 = pad[64:128, (PAD - 1 + 4 * i) * W:(PAD - 1 + 4 * i) * W + CH]
                s0, s1 = xc[0:64, :], xc[64:128, :]
            else:
                d0 = prr[0:64, 4 * i:4 * i + 4, PAD:PAD + W]
                d1 = prr[64:128, 4 * i:4 * i + 4, PAD - 1:PAD - 1 + W]
                s0 = xc[0:64, :].rearrange("p (h w) -> p h w", w=W)
                s1 = xc[64:128, :].rearrange("p (h w) -> p h w", w=W)
            nc.vector.tensor_scalar(d0, s0, av0, bv0, ALU.mult, ALU.add)
            nc.vector.tensor_scalar(d1, s1, av1, bv1, ALU.mult, ALU.add)

        # conv: per 512-px chunk, 10 tap-pair matmuls (K=128) + 1 single (K=64)
        cw = convw[br]
        for ci in range(NCHUNK):
            ps = ps_conv.tile([64, CH], F32, tag="conv")
            for g in range(10):
                if br == 0:
                    rhs = pad[:, (4 * ci + 2 * g) * W:(4 * ci + 2 * g) * W + CH]
                else:
                    rhs = prr[:, 4 * ci:4 * ci + 4, 2 * g:2 * g + W]
                nc.tensor.matmul(ps[:], cw[:, g * 64:(g + 1) * 64], rhs,
                                 start=(g == 0), stop=False)
            if br == 0:
                rhs = pad[0:64, (4 * ci + 20) * W:(4 * ci + 20) * W + CH]
            else:
                rhs = prr[0:64, 4 * ci:4 * ci + 4, 20:20 + W]
            nc.tensor.matmul(ps[:], cw[0:64, 640:704], rhs,
                             start=False, stop=True)
            nc.scalar.activation(scb[:, ci * CH:(ci + 1) * CH], ps[:], AF.Copy)

        # qkv pass 1: q|v (M=128), contiguous chunks -> cp_qv, (h,w)-major rows
        for ci in range(NCHUNK):
            ps = ps_qkv1.tile([128, CH], F32, tag="qkv1")
            nc.tensor.matmul(ps[:], qkv1[br],
                             scb[:, ci * CH:(ci + 1) * CH],
                             start=True, stop=True)
            ev = evp.tile([128, CH], F32, tag="ev")
            nc.scalar.activation(ev[:], ps[:], AF.Identity, bias=qkv1b[br])
            nc.sync.dma_start(cp_qv[br][:, ci * CH:(ci + 1) * CH], ev[:])

        # qkv pass 2: k (M=64), w-column chunks -> cp_k, (w,h)-major rows
        scr = scb[:].rearrange("p (h w) -> p w h", w=W)
        for ci in range(NCHUNK):
            ps = ps_qkv2.tile([64, CH], F32, tag="qkv2")
            nc.tensor.matmul(ps[:], qkv2[br],
                             scr[:, 4 * ci:4 * ci + 4, :],
                             start=True, stop=True)
            ev = evp.tile([128, CH], F32, tag="ev", name="ev2")[0:64, :]
            nc.scalar.activation(ev[:], ps[:], AF.Identity, bias=qkv2b[br])
            nc.sync.dma_start(cp_k[br][:, ci * CH:(ci + 1) * CH], ev[:])

    # release phase-1 pools (LIFO per space)
    ps_qkv2.release()
    ps_qkv1.release()
    ps_conv.release()
    pp.release()
    scp.release()

    # ---------------- phase 2: attention + projection ----------------
    scp2 = tc.alloc_tile_pool(name="scp2", bufs=1)
    s_cp = scp2.tile([128, HW], F32R, tag="s_cp")
    gsb = tc.alloc_tile_pool(name="gsb", bufs=1)
    g_sb = gsb.tile([128, 16 * 128], F32, tag="g_sb")
    sprq = tc.alloc_tile_pool(name="sprq", bufs=4)
    sprv = tc.alloc_tile_pool(name="sprv", bufs=4)
    sprk = tc.alloc_tile_pool(name="sprk", bufs=4)
    btev = tc.alloc_tile_pool(name="btev", bufs=2)
    sigp = tc.alloc_tile_pool(name="sigp", bufs=2)
    outp = tc.alloc_tile_pool(name="outp", bufs=2)
    ps_g = tc.alloc_tile_pool(name="ps_g", bufs=2, space="PSUM")
    ps_bt = tc.alloc_tile_pool(name="ps_bt", bufs=4, space="PSUM")
    ps_pj = tc.alloc_tile_pool(name="ps_pj", bufs=2, space="PSUM")

    # G matrices: gi=0 -> G_w = sum hq^T wv ; gi=1 -> G_h = sum wq^T hv
    for gi in range(2):
        qsrc = cp_qv[0] if gi == 0 else cp_qv[1]   # q lives in rows 0..64
        vsrc = cp_qv[1] if gi == 0 else cp_qv[0]   # v lives in rows 64..128
        for nh in range(NH):
            gps = ps_g.tile([128, 128], F32, tag="g")
            for d in range(D):
                c = nh * D + d
                qa = sprq.tile([128, 128], F32, tag="q")
                nc.sync.dma_start(
                    qa[:], qsrc[c:c + 1, :].rearrange("o (h w) -> o h w", w=W))
                va = sprv.tile([128, 128], F32, tag="v")
                nc.sync.dma_start(
                    va[:], vsrc[64 + c:65 + c, :].rearrange("o (h w) -> o h w", w=W))
                nc.tensor.matmul(gps[:], qa[:], va[:],
                                 start=(d == 0), stop=(d == D - 1))
            nc.scalar.activation(
                g_sb[:, (gi * NH + nh) * 128:(gi * NH + nh + 1) * 128],
                gps[:], AF.Copy)

    # B^T: per (branch, head, d): psum = k_t(nh,d) @ G -> natural [H,W] image,
    # reverse-spread into channel-partition S_cp (w_o rows 0-63, h_o rows 64-127)
    for gi in range(2):
        ksrc = cp_k[1] if gi == 0 else cp_k[0]  # w_o uses wk; h_o uses hk
        for nh in range(NH):
            gref = g_sb[:, (gi * NH + nh) * 128:(gi * NH + nh + 1) * 128]
            for d in range(D):
                c = nh * D + d
                ka = sprk.tile([128, 128], F32, tag="k")
                nc.sync.dma_start(
                    ka[:], ksrc[c:c + 1, :].rearrange("o (w h) -> o w h", h=H))
                bps = ps_bt.tile([128, 128], F32, tag="bt")
                nc.tensor.matmul(bps[:], ka[:], gref, start=True, stop=True)
                bt = btev.tile([128, 128], F32R, tag="btv")
                nc.scalar.activation(bt[:], bps[:], AF.Copy)
                nc.sync.dma_start(
                    s_cp[gi * 64 + c:gi * 64 + c + 1, :].rearrange(
                        "o (h w) -> o h w", w=W),
                    bt[:])

    # collapse the 128 reverse-spread DMA deps into one sync point so the
    # first projection matmul doesn't exceed the per-instruction wait limit
    tc.strict_bb_all_engine_barrier()

    # fused output projection (both branches, K=128) + sigmoid + x*sig -> y
    for ci in range(NCHUNK):
        pps = ps_pj.tile([64, CH], F32, tag="pj")
        nc.tensor.matmul(pps[:], projw, s_cp[:, ci * CH:(ci + 1) * CH],
                         start=True, stop=True)
        sg = sigp.tile([64, CH], F32, tag="sg")
        nc.scalar.activation(sg[:], pps[:], AF.Sigmoid, bias=projb)
        xc = outp.tile([64, CH], F32, tag="xm")
        nc.sync.dma_start(xc[:], a["x"][:, ci * CH:(ci + 1) * CH])
        ot = outp.tile([64, CH], F32, tag="ot")
        nc.vector.tensor_mul(ot[:], sg[:], xc[:])
        nc.sync.dma_start(a["y"][:, ci * CH:(ci + 1) * CH], ot[:])

    for p in (ps_pj, ps_bt, ps_g, outp, sigp, btev, sprk, sprv, sprq,
              gsb, scp2, evp, xcp, wp, dp):
        p.release()


def _prep_weights(inputs):
    """Host-side packing: BN affine, presummed conv taps, folded qkv biases."""
    inp = {k: np.asarray(v, dtype=np.float64) for k, v in inputs.items()}
    w = {}
    a1 = inp["bn1_g"] / np.sqrt(inp["bn1_v"] + EPS)
    b1 = inp["bn1_b"] - inp["bn1_m"] * a1
    a2 = inp["bn2_g"] / np.sqrt(inp["bn2_v"] + EPS)
    b2 = inp["bn2_b"] - inp["bn2_m"] * a2
    w["bnab"] = np.tile(np.stack([a1, b1, a2, b2], axis=1), (2, 1))  # [128, 4]

    def conv_pack(ws):
        eff = np.zeros((NTAP, C, C))
        for j, k in enumerate(KS):
            off = PAD - k // 2
            for i in range(k):
                eff[off + i] += ws[j][:, :, i]
        pk = np.zeros((128, 704))
        for g in range(10):
            pk[0:64, g * 64:(g + 1) * 64] = eff[2 * g].T
            pk[64:128, g * 64:(g + 1) * 64] = eff[2 * g + 1].T
        pk[0:64, 640:704] = eff[20].T
        return pk

    w["convw_h"] = conv_pack([inp[f"sc1_w{j}"][:, :, :, 0] for j in range(3)])
    w["convw_w"] = conv_pack([inp[f"sc2_w{j}"][:, :, 0, :] for j in range(3)])
    bch = inp["sc1_b0"] + inp["sc1_b1"] + inp["sc1_b2"]
    bcw = inp["sc2_b0"] + inp["sc2_b1"] + inp["sc2_b2"]

    scale = D * H ** (-0.5)
    idx = (np.arange(NH)[:, None] * 24 + np.arange(D)[None, :]).ravel()
    idx_q, idx_k, idx_v = idx, idx + 8, idx + 16

    for br, (qw, qb, bc) in enumerate(
            [(inp["hqkv_w"], inp["hqkv_b"], bch),
             (inp["wqkv_w"], inp["wqkv_b"], bcw)]):
        bfold = qb + qw @ bc
        Wq, Wk, Wv = qw[idx_q] * scale, qw[idx_k], qw[idx_v]
        bq, bk, bv = bfold[idx_q] * scale, bfold[idx_k], bfold[idx_v]
        sfx = "h" if br == 0 else "w"
        w[f"qkv1_{sfx}"] = np.concatenate([Wq.T, Wv.T], axis=1)        # [64,128]
        w[f"qkv1b_{sfx}"] = np.concatenate([bq, bv])[:, None]          # [128,1]
        w[f"qkv2_{sfx}"] = Wk.T                                        # [64,64]
        w[f"qkv2b_{sfx}"] = bk[:, None]                                # [64,1]

    w["projw"] = np.concatenate([inp["wout_w"].T, inp["hout_w"].T], axis=0)  # [128,64]
    w["projb"] = (inp["wout_b"] + inp["hout_b"])[:, None]                    # [64,1]

    w128 = np.zeros((128, 1478))
    w128[:, 0:704] = w["convw_h"]
    w128[:, 704:1408] = w["convw_w"]
    w128[:, 1408:1472] = w["projw"]
    w128[:, 1472:1473] = w["qkv1b_h"]
    w128[:, 1473:1474] = w["qkv1b_w"]
    w128[:, 1474:1478] = w["bnab"]
    w64 = np.zeros((64, 387))
    w64[:, 0:128] = w["qkv1_h"]
    w64[:, 128:256] = w["qkv1_w"]
    w64[:, 256:320] = w["qkv2_h"]
    w64[:, 320:384] = w["qkv2_w"]
    w64[:, 384:385] = w["qkv2b_h"]
    w64[:, 385:386] = w["qkv2b_w"]
    w64[:, 386:387] = w["projb"]
    return {"w128": _to_f32r(w128), "w64": _to_f32r(w64)}


_NC_CACHE = {}
_RUN_OPTS = {"trace": False}
_LAST_RESULT = {}

_W_SHAPES = {"x": [C, HW], "w128": [128, 1478], "w64": [64, 387]}
_W_DTYPES = {"x": F32, "w128": F32R, "w64": F32R}


def _to_f32r(a):
    """fp32 -> fp32r: round mantissa to 11 bits (top 20 bits kept)."""
    u = np.ascontiguousarray(a, dtype=np.float32).view(np.uint32).astype(np.uint64)
    u = (u + 0x800) & np.uint64(0xFFFFF000)
    return u.astype(np.uint32).view(np.float32)


def _build_nc(reps=1):
    key = f"nc{reps}"
    if key in _NC_CACHE:
        return _NC_CACHE[key]
    nc = bacc.Bacc(trn_type="TRN2", target_bir_lowering=False, debug=False)
    a = {}
    for n, s in _W_SHAPES.items():
        a[n] = nc.dram_tensor(n, s, _W_DTYPES[n], kind="ExternalInput").ap()
    a["y"] = nc.dram_tensor("y", [C, HW], F32, kind="ExternalOutput").ap()
    with tile.TileContext(nc) as tc:
        _kernel_body(tc, a, reps=reps)
    nc.compile()
    _NC_CACHE[key] = nc
    return nc


def _in_maps(inputs):
    w = _prep_weights(inputs)
    x = np.ascontiguousarray(np.asarray(inputs["x"], dtype=np.float32))
    maps = []
    for core in range(N_CORES):
        m = {"x": np.ascontiguousarray(x[core].reshape(C, HW))}
        m.update(w)
        maps.append(m)
    return maps


def kernel(**inputs):
    from concourse.bass_utils import run_bass_kernel_spmd

    nc = _build_nc()
    res = run_bass_kernel_spmd(nc, _in_maps(inputs), core_ids=list(range(N_CORES)),
                               trace=_RUN_OPTS["trace"])
    _LAST_RESULT["res"] = res
    out = np.stack([res.results[i]["y"].reshape(C, H, W) for i in range(N_CORES)])
    return out.astype(np.float32)


if __name__ == "__main__":
    nc = _build_nc()
    print("built ok")
